# revision 1
# baseline (speedup 1.0000x reference)
"""Trainium2 Bass kernel for a 1-D correlation volume (stereo cost volume).

Problem: out[n, i, h, w] = (1/C) * sum_c x1[n,c,h,w] * x2[n,c,h,w-i],
zero where w-i < 0, for i in 0..D (D=64).
Shapes: x1, x2 = [8, 128, 128, 256] f32; out = [8, 65, 128, 256] f32.

Sharding: data-parallel over the batch dim — each of the 8 NeuronCores
processes one sample end to end (no collectives).

Per-core algorithm
------------------
The contraction over c maps onto the TensorEngine as a banded Gram
matmul: for each (h, w-tile ts) the matmul
    band[p, col] = (1/C) * sum_c x1[c, ts+p] * x2[c, (ts-64)+col]
holds every needed output as band[p, p + 64 - i].  Extracting those 65
diagonals cannot be done by any SBUF compute engine (per-partition
varying offsets), so the band is staged to a DRAM scratch where linear
addressing collapses the diagonal into a plain 3-dim strided DMA:
    addr(h, p, j) = h*192 + p*(H*192 + 1) + j       (j = 64 - i)
which reads back with h in the partition dim.  A small DVE repack then
reverses j -> i and transposes (p, j) -> (i, p) so the final store to
out[n, :, :, ts:ts+128] is a contiguous 3-dim DMA.
"""

import numpy as np

import concourse.bass as bass
import concourse.tile as tile
from concourse import bacc, mybir
from concourse.bass_utils import run_bass_kernel_spmd

# Problem constants (hardcoded per the harness contract).
B = 8          # batch == number of cores
C = 128        # channels (matmul K)
H = 128        # rows
W = 256        # cols
D = 64         # max disparity
ND = D + 1     # number of disparities (65)
T = 128        # w-tile size (matmul M)
NT = W // T    # 2 w-tiles
BANDC = T + D  # 192 band columns per tile
HB = 8         # h rows per load/staging block
PH = 64        # p-half size for the extraction stage

F32 = mybir.dt.float32


def _corr_body(tc, out_d, x1_d, x2_d):
    nc = tc.nc
    with (
        tc.tile_pool(name="io", bufs=2) as io_pool,
        tc.tile_pool(name="band", bufs=2) as band_pool,
        tc.tile_pool(name="psum", bufs=8, space="PSUM") as psum_pool,
        tc.tile_pool(name="fib", bufs=2) as fib_pool,
        tc.tile_pool(name="dram", bufs=1, space="DRAM") as dram_pool,
    ):
        # DRAM scratch, one band volume per w-tile: scr[p, h, col].
        scr = [
            dram_pool.tile([T, H, BANDC], F32, tag=f"scr{t}", name=f"scr{t}")
            for t in range(NT)
        ]

        for hb in range(0, H, HB):
            x1t = io_pool.tile([C, HB * W], F32, tag="x1t")
            nc.sync.dma_start(x1t[:], x1_d[:, hb : hb + HB, :])
            x2t = io_pool.tile([C, HB * W], F32, tag="x2t")
            nc.sync.dma_start(x2t[:], x2_d[:, hb : hb + HB, :])

            bb = [
                band_pool.tile([T, HB * BANDC], F32, tag=f"bb{t}", name=f"bb{t}")
                for t in range(NT)
            ]
            # ts=0 band columns 0:64 are w' < 0 -> zero padding.
            nc.gpsimd.memset(bb[0][:], 0.0)

            for hl in range(HB):
                base = hl * W
                # w-tile 0: band cols 64:192 <- x1[:, 0:128]^T @ x2[:, 0:128]
                pt0 = psum_pool.tile([T, T], F32, tag="pt")
                nc.tensor.matmul(
                    pt0[:],
                    x1t[:, base : base + T],
                    x2t[:, base : base + T],
                    start=True,
                    stop=True,
                )
                nc.scalar.mul(
                    bb[0][:, hl * BANDC + D : (hl + 1) * BANDC], pt0[:], 1.0 / C
                )
                # w-tile 1: band cols 0:192 <- x1[:, 128:256]^T @ x2[:, 64:256]
                pt1 = psum_pool.tile([T, BANDC], F32, tag="pt")
                nc.tensor.matmul(
                    pt1[:],
                    x1t[:, base + T : base + 2 * T],
                    x2t[:, base + T - D : base + W],
                    start=True,
                    stop=True,
                )
                nc.scalar.mul(
                    bb[1][:, hl * BANDC : (hl + 1) * BANDC], pt1[:], 1.0 / C
                )

            for t in range(NT):
                nc.sync.dma_start(scr[t][:, hb : hb + HB, :], bb[t][:])

        # Extraction: diagonal fibers out of the scratch, h in partitions.
        for t in range(NT):
            for ph in range(0, T, PH):
                ft = fib_pool.tile([H, PH, ND], F32, tag="ft")
                src = bass.AP(
                    scr[t].tensor,
                    scr[t].offset + ph * (H * BANDC + 1),
                    [[BANDC, H], [H * BANDC + 1, PH], [1, ND]],
                )
                nc.sync.dma_start(ft[:], src)

                gt = fib_pool.tile([H, ND, PH], F32, tag="gt")
                for j in range(ND):
                    nc.vector.tensor_copy(gt[:, D - j, :], ft[:, :, j])

                dst = bass.AP(
                    out_d,
                    t * T + ph,
                    [[W, H], [H * W, ND], [1, PH]],
                )
                nc.sync.dma_start(dst, gt[:])


_NC_CACHE = None


def _build_nc():
    global _NC_CACHE
    if _NC_CACHE is not None:
        return _NC_CACHE
    nc = bacc.Bacc("TRN2")
    x1_d = nc.declare_dram_parameter("x1", [C, H, W], F32, isOutput=False)
    x2_d = nc.declare_dram_parameter("x2", [C, H, W], F32, isOutput=False)
    out_d = nc.declare_dram_parameter("out", [ND, H, W], F32, isOutput=True)
    with tile.TileContext(nc) as tc:
        _corr_body(tc, out_d, x1_d, x2_d)
    nc.finalize()
    _NC_CACHE = nc
    return nc


def kernel(x1: np.ndarray, x2: np.ndarray) -> np.ndarray:
    assert x1.shape == (B, C, H, W) and x2.shape == (B, C, H, W)
    nc = _build_nc()
    in_maps = [
        {
            "x1": np.ascontiguousarray(x1[n], dtype=np.float32),
            "x2": np.ascontiguousarray(x2[n], dtype=np.float32),
        }
        for n in range(B)
    ]
    res = run_bass_kernel_spmd(nc, in_maps, list(range(B)))
    return np.stack([res.results[n]["out"] for n in range(B)], axis=0)

